# revision 54
# baseline (speedup 1.0000x reference)
"""Multi-head attention (B=4, S=2048, C=768, H=8, HD=96) on 8 TRN2 NeuronCores.

Strategy: tensor-parallel by head — one head per core. All TensorEngine
matmuls run bf16 inputs with f32 PSUM accumulation.

  - q/k are computed transposed: qT/kT [HD, tok] = W_chunk.T @ xT with the
    weight chunk stationary and 512-token moving operand; xT and all weights
    are host pre-transposed bf16.
  - v is computed directly in k-major layout (out [tok, HD] = xT_chunk.T @ Wv)
    so no DMA/PE transposes of v are needed; a ones column at hd index 96
    makes PV accumulate the softmax denominator in partition row 96.
  - RoPE runs in the transposed layout: the pair-swap is a DVE stream
    shuffle; the sign lives in the sin table (s[2i] = -sin[2i]).
  - Attention per (b, q-tile): scores.T [k,q] = kT.T @ qT on PE, exp on ACT
    (scale folded in; no max-subtraction needed: scores ~ N(0,1)), P.T (bf16)
    feeds PV directly: acc [HD+1, q] = v_aug.T @ P.T.
  - STRIDED q-tiles: q-tile g covers tokens {256j + 64g + t : j<8, t<64} of
    its batch (RoPE scatter-writes a qt-major buffer), so each q-tile
    completes a full 64-token column slice of the exchange payload for every
    destination. Batches 0-2 exchange once per batch; batch 3 in two halves,
    so the tail only exposes a half-size collective.
  - A dummy 1KB AllToAll fires at t=0: the CC stream's ~30-65us cold start
    runs under the batch-0 prologue instead of wedging the first exchange.
  - Normalized outputs stage in SBUF ([hd, dst, qt, t]) and store with one
    contiguous 512B-run DMA per batch — fragmented exchange buffers slow the
    CC-core reads by 2-3x.
  - exp runs N=1024 (one ACTIVATE per pair of score banks), amortizing the
    ~352-cycle ACT instruction overhead.
  - Normalization: approx-reciprocal of the denominator row on DVE, then a
    partition_broadcast on the otherwise-idle Pool engine (keeping it out of
    the PE/PSUM path avoids ring couplings that convoy the pipeline).
  - QKV matmuls for batch b+1 (and hoisted tail projections) are injected as
    PE "fillers" between the score/PV matmuls of batch b so the PE never
    idles waiting for exp. Projections run at the tail, 2-bank interleaved;
    proj(3) goes in column halves so the first half hoists into batch-3
    attention as soon as its half-collective lands.
  - PSUM (8 banks): score pairs 2x[128,2,512] + PV acc 2 + qkv 2 (q/k/v
    chains time-share via the pool ring).
  - Queue discipline: agc loads ride gpsimd (a collective-done wait on Sync
    wedges the store stream and stalls the pipeline through the tile rings);
    proj outputs go out via the ACT queue. The prologue keeps the DMA
    instruction count on the critical path minimal (~0.6-0.75us issue cost
    per DMA instruction).
"""

import numpy as np
from contextlib import ExitStack

import concourse.bass as bass
from concourse import bacc
import concourse.tile as tile
from concourse import mybir
from concourse.bass_utils import run_bass_kernel_spmd

B, S, C, H, HD = 4, 2048, 768, 8, 96
T = B * S            # 8192 tokens
NCORES = 8
TSLICE = T // NCORES  # 1024 tokens per core for the projection
BSLICE = S // NCORES  # 256 tokens per (core, batch)
KC = C // 128        # 6 contraction chunks of 128
F32 = mybir.dt.float32
BF16 = mybir.dt.bfloat16


def build_nc():
    nc = bacc.Bacc(None, num_devices=NCORES)

    xT = nc.declare_dram_parameter("xT", [C, T], BF16, isOutput=False)
    wqkvT = nc.declare_dram_parameter("wqkvT", [C, 3 * HD], BF16, isOutput=False)
    wprojT = nc.declare_dram_parameter("wprojT", [C, C], BF16, isOutput=False)
    cosT = nc.declare_dram_parameter("cosT", [HD, S], BF16, isOutput=False)
    sT = nc.declare_dram_parameter("sT", [HD, S], BF16, isOutput=False)
    biasd = nc.declare_dram_parameter("bias", [128, KC], F32, isOutput=False)
    outd = nc.declare_dram_parameter("out", [C, TSLICE], BF16, isOutput=True)

    # exchange buffers: [8 dst blocks x HD, tokens]. Batches 0-2 exchange
    # once per batch (their collectives hide under compute); batch 3 goes in
    # two halves so the final exposed collective is half-sized. Each strided
    # q-tile writes the 64-token column slice it completes.
    a2a_in = [nc.dram_tensor(f"a2a_in{b}", [C, BSLICE], BF16) for b in range(3)]
    a2a_out = [nc.dram_tensor(f"a2a_out{b}", [C, BSLICE], BF16) for b in range(3)]
    # batch 3 splits 192+64 so the only tail-exposed collective carries 64
    # tokens (~98KB); batch 0 splits 128+128 so its exchange completes ~20us
    # earlier and the agc(0) collective-done wait can never wedge the gpsimd
    # queue mid-pipeline. More chunks than that lose: each collective has a
    # ~7us floor and the chunks serialize on the CC stream.
    B3W = (192, 64)
    a2a_in3 = [nc.dram_tensor(f"a2a_in3h{i}", [C, w], BF16)
               for i, w in enumerate(B3W)]
    a2a_out3 = [nc.dram_tensor(f"a2a_out3h{i}", [C, w], BF16)
                for i, w in enumerate(B3W)]
    a2a_in0 = [[nc.dram_tensor(f"a2a_in{b}h{i}", [C, 128], BF16)
                for i in range(2)] for b in range(2)]
    a2a_out0 = [[nc.dram_tensor(f"a2a_out{b}h{i}", [C, 128], BF16)
                 for i in range(2)] for b in range(2)]
    # full-size warmup: a 1KB dummy doesn't establish the large-transfer
    # path, and the first real exchange then still runs 2-3x slow
    warm_in = nc.dram_tensor("warm_in", [C, BSLICE], BF16)
    warm_out = nc.dram_tensor("warm_out", [C, BSLICE], BF16)

    SCALE = HD ** -0.5
    MULT = mybir.AluOpType.mult
    ADD = mybir.AluOpType.add
    EXP = mybir.ActivationFunctionType.Exp

    with tile.TileContext(nc, num_cores=NCORES) as tc, ExitStack() as ctx:
        const = ctx.enter_context(tc.tile_pool(name="const", bufs=1))
        # bufs=2 doubles as the prologue bandwidth gate: groups 2-3 (and
        # every later prefetch) queue behind consumption instead of
        # round-robin-stealing HBM bandwidth from the group-0/1 loads that
        # gate the first matmuls
        xtp = ctx.enter_context(tc.tile_pool(name="xtp", bufs=2))
        rawp = ctx.enter_context(tc.tile_pool(name="rawp", bufs=2))
        ropep = ctx.enter_context(tc.tile_pool(name="ropep", bufs=2))
        Pp = ctx.enter_context(tc.tile_pool(name="Pp", bufs=4))
        rcp = ctx.enter_context(tc.tile_pool(name="rcp", bufs=2))
        yp = ctx.enter_context(tc.tile_pool(name="yp", bufs=2))
        agcp = ctx.enter_context(tc.tile_pool(name="agcp", bufs=4))

        # PSUM (8 banks): qkv 2 + score pairs 2x2 + PV acc 2. The norm
        # broadcast borrows a slot in the score ring between q-tiles.
        psqkv = ctx.enter_context(tc.tile_pool(name="psqkv", bufs=2, space="PSUM"))
        pssc = ctx.enter_context(tc.tile_pool(name="pssc", bufs=2, space="PSUM"))
        psacc = ctx.enter_context(tc.tile_pool(name="psacc", bufs=2, space="PSUM"))

        # dummy collective with no input deps, fired at t=0: the CC stream
        # pays its ~30us cold-start here, under the batch-0 prologue, so the
        # first real exchange runs at the steady ~8us and nothing queued
        # behind its done-semaphore wedges mid-pipeline
        nc.gpsimd.collective_compute(
            "AllToAll", mybir.AluOpType.bypass,
            replica_groups=[list(range(NCORES))],
            ins=[warm_in.ap().opt()],
            outs=[warm_out.ap().opt()],
        )

        # --- constants --- (wq + first x groups first: they gate the PE
        # start; the DMA rings drain all queued transfers round-robin, so
        # everything not needed immediately is issued later / behind an
        # xtp-slot gate)
        # every DMA instruction costs ~0.6-0.75us of queue-issue time, so the
        # prologue keeps the count on the critical path minimal: kc0 of the
        # weights and of x group 0 first (2 small DMAs), then the rest whole
        wq_sb = const.tile([128, KC, 3 * HD], BF16)
        wqv = wqkvT.ap().rearrange("(kc p) n -> p kc n", p=128)
        nc.sync.dma_start(wq_sb[:, 0, :], wqv[:, 0, :])
        xTv = xT.ap().rearrange("(kc p) t -> p kc t", p=128)  # [128, KC, T]
        cosT_sb = const.tile([HD, S], BF16)
        sT_sb = const.tile([HD, S], BF16)
        wp_sb = const.tile([128, KC, C], BF16)
        bias_sb = const.tile([128, KC], F32)

        def load_tail_consts():
            nc.scalar.dma_start(
                wp_sb, wprojT.ap().rearrange("(kc p) n -> p kc n", p=128))
            nc.scalar.dma_start(bias_sb, biasd.ap())

        # persistent ping/pong per-batch qT stored qt-MAJOR ([HD, qt, 512]
        # with tile qt holding tokens {256j + 64qt + t} in (j, t) column
        # order — RoPE scatter-writes it), per-group kT, and k-major v with
        # a ones column at hd index 96 (denominator accumulator).
        qT = [const.tile([HD, 4, 512], BF16, name=f"qT{i}") for i in range(2)]
        kT = [[const.tile([HD, 512], BF16, name=f"kT{i}g{g}") for g in range(4)]
              for i in range(2)]
        vA = [[const.tile([128, 4, HD + 1], BF16, name=f"vA{i}g{g}")
               for g in range(4)] for i in range(2)]
        for i in range(2):
            for g in range(4):
                nc.vector.memset(vA[i][g][:, :, HD:HD + 1], 1.0)

        ones_sb = const.tile([1, HD], BF16, name="ones_sb")
        nc.vector.memset(ones_sb, 1.0)

        # per-batch normalized-output staging [hd, dst j, qt, t]: each qt's
        # norm writes its strided slice; one contiguous 512B-run store per
        # batch (b3: per half) keeps the exchange buffers DRAM-friendly
        stage = [const.tile([HD, 8, 4, 64], BF16, name=f"stage{i}")
                 for i in range(2)]

        SWAPMASK = []
        for i in range(16):
            SWAPMASK += [2 * i + 1, 2 * i]

        def load_xtc(b, g, split=False, eng=None):
            # split=True (prologue group 0): kc0 alone, then the rest — the
            # first matmuls start after ~0.6MB instead of the full group
            tok0 = b * S + g * 512
            xtc = xtp.tile([128, KC, 512], BF16, tag="xtc")
            e = eng or nc.sync
            if split:
                e.dma_start(xtc[:, 0, :], xTv[:, 0, tok0:tok0 + 512])
                e.dma_start(xtc[:, 1:, :], xTv[:, 1:, tok0:tok0 + 512])
            else:
                e.dma_start(xtc, xTv[:, :, tok0:tok0 + 512])
            return xtc

        def qkv_fillers(b, xtcs, alt=False):
            """Filler closures computing q/k/v for batch b (4 groups).

            With 2 qkv PSUM banks the chains time-share the pool ring:
            q(slot a), k(slot b), then v waits for rope-q's copy to free
            slot a; the next group's q waits for rope-k's copy, etc.
            alt=True (batch-0 prologue only): odd groups borrow the then-idle
            score banks so two group-chains run in parallel.
            """
            out = []
            q_b, k_b, v_b = qT[b % 2], kT[b % 2], vA[b % 2]
            for g in range(4):
                state = {}
                xtc = xtcs[g]
                seq = slice(g * 512, (g + 1) * 512)
                pool, ptag = ((pssc, "sc") if (alt and g % 2 == 1)
                              else (psqkv, "qkv"))

                def alloc_qk(state=state, pool=pool, ptag=ptag):
                    state["q"] = pool.tile([HD, 512], F32, tag=ptag, name="psq")
                    state["k"] = pool.tile([HD, 512], F32, tag=ptag, name="psk")

                def alloc_v(state=state, pool=pool, ptag=ptag):
                    state["v"] = pool.tile([128, 4, HD], F32, tag=ptag, name="psv")

                def mm_qk(kc, which, state=state, xtc=xtc):
                    nc.tensor.matmul(
                        state[which],
                        wq_sb[:, kc, (0 if which == "q" else HD):(HD if which == "q" else 2 * HD)],
                        xtc[:, kc, :],
                        start=(kc == 0), stop=(kc == KC - 1),
                    )

                def mm_v(kc, c, state=state, xtc=xtc):
                    nc.tensor.matmul(
                        state["v"][:, c, :],
                        xtc[:, kc, c * 128:(c + 1) * 128],
                        wq_sb[:, kc, 2 * HD:3 * HD],
                        start=(kc == 0), stop=(kc == KC - 1),
                    )

                def rope(which, dst, dst_scatter, state=state, seq=seq):
                    # bf16 intermediates keep the two multiplies and the add
                    # in the DVE 2x perf mode (fp32 outputs force 1x)
                    ps = state[which]
                    raw = rawp.tile([HD, 512], BF16, tag="raw")
                    nc.vector.tensor_copy(out=raw, in_=ps)
                    rot = rawp.tile([HD, 512], BF16, tag="rot")
                    nc.vector.stream_shuffle(rot, raw, SWAPMASK)
                    t1 = ropep.tile([HD, 512], BF16, tag="t1")
                    nc.vector.tensor_tensor(t1, raw, cosT_sb[:, seq], MULT)
                    t2 = ropep.tile([HD, 512], BF16, tag="t2")
                    nc.vector.tensor_tensor(t2, rot, sT_sb[:, seq], MULT)
                    if dst_scatter:
                        t1 = t1.rearrange("hd (j q t) -> hd j q t", j=2, t=64)
                        t2 = t2.rearrange("hd (j q t) -> hd j q t", j=2, t=64)
                    nc.vector.tensor_tensor(dst, t1, t2, ADD)

                def copy_v(g=g, state=state, v_b=v_b):
                    for c in range(4):
                        nc.vector.tensor_copy(
                            out=v_b[g][:, c, 0:HD], in_=state["v"][:, c, :])

                out.append(alloc_qk)
                for kc in range(KC):
                    out.append(lambda kc=kc, f=mm_qk: f(kc, "q"))
                    out.append(lambda kc=kc, f=mm_qk: f(kc, "k"))
                # rope-q scatter-writes the qt-major buffer: group g's
                # token col (jr, qt, t) -> tile qt, col 64*(2g+jr) + t.
                # Its first copy frees the q PSUM slot for v.
                qdst = q_b.rearrange("hd q (j t) -> hd j q t", j=8, t=64) \
                    [:, 2 * g:2 * g + 2, :, :]
                out.append(lambda f=rope, qdst=qdst: f("q", qdst, True))
                out.append(alloc_v)
                # v regions share one PSUM bank: each chunk c must fully
                # accumulate (start..stop) before the next chunk's start
                vs = []
                for c in range(4):
                    for kc in range(KC):
                        vs.append(lambda kc=kc, c=c, f=mm_v: f(kc, c))
                vs.append(lambda f=rope, k_b=k_b, g=g: f("k", k_b[g][:, :], False))
                out.extend(vs)
                out.append(copy_v)
            return out

        def load_agc_only(b, eng=None):
            # always gpsimd unless told otherwise: a collective-done wait on
            # the Sync queue convoys the per-tile a2a stores behind it and
            # stalls the whole pipeline through the onorm tile ring.
            agc = agcp.tile([128, KC, BSLICE], BF16, tag="agc", name="agc")
            e = eng or nc.gpsimd
            if b < 2:
                for i in range(2):
                    e.dma_start(
                        agc[:, :, 128 * i:128 * (i + 1)],
                        a2a_out0[b][i].ap().rearrange("(kc p) t -> p kc t", p=128))
            elif b < 3:
                e.dma_start(
                    agc, a2a_out[b].ap().rearrange("(kc p) t -> p kc t", p=128))
            else:
                t0 = 0
                for i, w in enumerate(B3W):
                    e.dma_start(
                        agc[:, :, t0:t0 + w],
                        a2a_out3[i].ap().rearrange("(kc p) t -> p kc t", p=128))
                    t0 += w
            return agc

        def proj_fillers(b, wide=False, agc_pre=None, cols=(0, BSLICE)):
            """Filler closures projecting batch b's gathered attention output.

            wide=True (tail only): interleave 2 ko-chunk accumulators across
            the idle qkv PSUM banks so consecutive matmuls never share a
            bank. Bias-add + f32 copy runs on DVE to keep ACT free for exp.
            cols: token sub-range — batch 3 projects in halves so the first
            half hoists into batch-3 attention once its collective lands.
            """
            out = []
            state = {}
            t0, t1 = cols
            tsz = t1 - t0

            def load_agc():
                # tail-only path: the ACT queue is safe here (all exps are
                # past; only this proj's own downstream sits behind it), and
                # HWDGE issue is ~6-10us faster than SWDGE descriptor gen
                state["agc"] = agc_pre if agc_pre is not None else \
                    load_agc_only(b, eng=nc.scalar)
                state["y"] = yp.tile([128, KC, tsz], BF16, tag="y", name="ysb")

            def alloc_py(ko, pool, name):
                state[ko] = pool.tile([128, tsz], F32, tag="qkv" if pool is psqkv else "py", name=name)

            def mm(ko, kc):
                nc.tensor.matmul(
                    state[ko], wp_sb[:, kc, ko * 128:(ko + 1) * 128],
                    state["agc"][:, kc, t0:t1],
                    start=(kc == 0), stop=(kc == KC - 1),
                )

            def biascopy(ko):
                nc.vector.tensor_scalar_add(
                    state["y"][:, ko, :], state[ko], bias_sb[:, ko:ko + 1])

            def store():
                nc.scalar.dma_start(
                    outd.ap().rearrange("(kc p) t -> p kc t", p=128)
                    [:, :, b * BSLICE + t0:b * BSLICE + t1],
                    state["y"])

            out.append(load_agc)
            if wide:
                for grp in range(3):
                    kos = [2 * grp + i for i in range(2)]
                    for ko in kos:
                        out.append(lambda ko=ko: alloc_py(ko, psqkv, "pyw"))
                    for kc in range(KC):
                        for ko in kos:
                            out.append(lambda ko=ko, kc=kc: mm(ko, kc))
                    for ko in kos:
                        out.append(lambda ko=ko: biascopy(ko))
            else:
                for ko in range(KC):
                    out.append(lambda ko=ko: alloc_py(ko, pspy, "py"))
                    for kc in range(KC):
                        out.append(lambda ko=ko, kc=kc: mm(ko, kc))
                    out.append(lambda ko=ko: biascopy(ko))
            out.append(store)
            return out

        def attention_tile(b, qt, fillers):
            q_b, k_b, v_b = qT[b % 2], kT[b % 2], vA[b % 2]
            # qt-major strided q-tile: tokens {256j + 64*qt + t}, (j,t) order
            qmv = q_b[:, qt, :]

            def emit(n):
                for _ in range(min(n, len(fillers))):
                    fillers.pop(0)()

            acc = psacc.tile([HD + 1, 512], F32, tag="acc")
            Pts = [None] * 8
            for i in range(8):
                sc2 = pssc.tile([128, 2, 512], F32, tag="sc")
                for h2 in range(2):
                    kt = 2 * i + h2
                    nc.tensor.matmul(
                        sc2[:, h2, :],
                        k_b[kt // 4][:, (kt % 4) * 128:(kt % 4 + 1) * 128],
                        qmv,
                        start=True, stop=True,
                    )
                # one exp instruction covers both score banks: N=1024
                # amortizes the ~352-cycle ACT instruction overhead
                Pt2 = Pp.tile([128, 2, 512], BF16, tag="Pt")
                nc.scalar.activation(
                    Pt2.rearrange("p two n -> p (two n)"),
                    sc2.rearrange("p two n -> p (two n)"), EXP, scale=SCALE)
                Pts[i] = Pt2
                if i > 0:
                    for h2 in range(2):
                        kt = 2 * (i - 1) + h2
                        nc.tensor.matmul(
                            acc, v_b[kt // 4][:, kt % 4, :], Pts[i - 1][:, h2, :],
                            start=(kt == 0), stop=False,
                        )
                emit(8)
            for h2 in range(2):
                kt = 14 + h2
                nc.tensor.matmul(
                    acc, v_b[3][:, kt % 4, :], Pts[7][:, h2, :],
                    start=False, stop=(kt == 15),
                )

            # normalize: approx-reciprocal of the denominator row (single
            # custom-DVE op, ~18 bits), broadcast across partitions on the
            # otherwise-idle Pool engine. Keeping the broadcast out of the
            # score PSUM ring matters: a bc tile there couples next-tile
            # scores to the DVE queue depth and convoys the whole pipeline.
            dnrow = rcp.tile([1, 512], F32, tag="dnrow")
            nc.vector.tensor_copy(out=dnrow, in_=acc[HD:HD + 1, :])
            rc = rcp.tile([1, 512], F32, tag="rc")
            nc.vector.reciprocal_approx_fast(out=rc, in_=dnrow)
            bcs = rcp.tile([HD, 512], F32, tag="bcs", name="bcs")
            nc.gpsimd.partition_broadcast(bcs, rc, channels=HD)
            # normalized tile lands in the staging buffer at its qt slice,
            # [hd, j, t] (strided DVE write)
            stg = stage[b % 2]
            nc.vector.tensor_tensor(
                stg[:, :, qt, :],
                acc[0:HD, :].rearrange("hd (j t) -> hd j t", t=64),
                bcs.rearrange("hd (j t) -> hd j t", t=64), MULT)
            # batch 2: one contiguous store + AllToAll after its last tile.
            # Batches 0-1: two halves each (early completion keeps the
            # agc(0)/agc(1) collective-done waits from wedging the gpsimd
            # queue mid-pipeline). Batch 3: 192+64 (small exposed tail).
            chunk = None
            if b < 2 and qt in (1, 3):
                chunk = (a2a_in0[b][qt // 2], a2a_out0[b][qt // 2],
                         (0, 2) if qt == 1 else (2, 4))
            elif b == 2 and qt == 3:
                chunk = (a2a_in[b], a2a_out[b], (0, 4))
            elif b == 3 and qt in (2, 3):
                chunk = (a2a_in3[qt - 2], a2a_out3[qt - 2],
                         (0, 3) if qt == 2 else (3, 4))
            if chunk is not None:
                cin, cout, (g0, g1) = chunk
                nc.sync.dma_start(
                    cin.ap().rearrange("(j hd) t -> hd j t", hd=HD),
                    stg[:, :, g0:g1, :].rearrange("hd j g t -> hd j (g t)"))
                nc.gpsimd.collective_compute(
                    "AllToAll", mybir.AluOpType.bypass,
                    replica_groups=[list(range(NCORES))],
                    ins=[cin.ap().opt()],
                    outs=[cout.ap().opt()],
                )

        # prologue: batch 0 qkv straight-line; x groups ride the sync and
        # scalar DMA rings in parallel so the first matmuls start as soon as
        # group 0 chunk 0 lands. cos/sin lead the scalar ring: rope(g0)
        # needs them ~6us in, and a late arrival stalls the 2 qkv PSUM banks.
        nc.scalar.dma_start(cosT_sb, cosT.ap())
        nc.scalar.dma_start(sT_sb, sT.ap())
        xtcs0 = [load_xtc(0, 0, split=True)]
        nc.sync.dma_start(wq_sb[:, 1:, :], wqv[:, 1:, :])
        xtcs0.append(load_xtc(0, 1, eng=nc.scalar))
        xtcs0.append(load_xtc(0, 2))
        xtcs0.append(load_xtc(0, 3, eng=nc.scalar))
        load_tail_consts()
        for f in qkv_fillers(0, xtcs0):
            f()

        # all projections run at the tail (3-bank wide): their agc loads sit
        # on the gpsimd queue after every trigger, so collective-done waits
        # can never convoy per-tile work
        PROJ_IN = {}
        agc_pre = {}
        for b in range(B):
            fillers = []
            if b + 1 < B:
                xtcs = [load_xtc(b + 1, g) for g in range(4)]
                fillers += qkv_fillers(b + 1, xtcs)
            for pb in PROJ_IN.get(b, []):
                fillers += proj_fillers(pb)
            for qt in range(4):
                attention_tile(b, qt, fillers)
                if b == B - 1 and qt == 0:
                    # preload so the scheduler can hoist tail projections
                    # into batch 3's exp-paced PE gaps. Both ride gpsimd (a
                    # collective wait on Sync wedges the store stream), and
                    # only after batch 3's first tile so their waits sit
                    # behind that broadcast — never ahead of live per-tile
                    # work while an old collective is still in flight.
                    agc_pre[0] = load_agc_only(0)
                    agc_pre[1] = load_agc_only(1)
            for f in fillers:
                f()
        # tail: proj(0..2) overlap the last collectives; proj(3) goes in
        # halves so its first 128 tokens hoist into batch-3 attention as
        # soon as the first half-collective lands. All 2-bank interleaved
        # across the now-idle qkv PSUM banks.
        for pb in range(3):
            for f in proj_fillers(pb, wide=True, agc_pre=agc_pre.get(pb)):
                f()
        agc3 = load_agc_only(3, eng=nc.scalar)
        for cols in ((0, 192), (192, 256)):
            for f in proj_fillers(3, wide=True, agc_pre=agc3, cols=cols):
                f()

    nc.compile()
    return nc


_NC_CACHE = None


def _get_nc():
    global _NC_CACHE
    if _NC_CACHE is None:
        _NC_CACHE = build_nc()
    return _NC_CACHE


def make_in_maps(x, cos, sin, Wqkv, Wproj, bproj):
    import ml_dtypes

    bf16 = ml_dtypes.bfloat16
    x = np.asarray(x, np.float32)
    cos = np.asarray(cos, np.float32)
    sin = np.asarray(sin, np.float32)
    Wqkv = np.asarray(Wqkv, np.float32)
    Wproj = np.asarray(Wproj, np.float32)
    bproj = np.asarray(bproj, np.float32)

    xT = np.ascontiguousarray(x.reshape(T, C).T.astype(bf16))  # [C, T] bf16
    wprojT = np.ascontiguousarray(Wproj.T.astype(bf16))        # [C_in, C_out]
    s = sin.copy()
    s[:, 0::2] = -sin[:, 0::2]
    cosT = np.ascontiguousarray(cos.T.astype(bf16))            # [HD, S] bf16
    sT = np.ascontiguousarray(s.T.astype(bf16))                # [HD, S] bf16
    bias2 = np.ascontiguousarray(bproj.reshape(KC, 128).T)     # [128, KC]

    in_maps = []
    for h in range(NCORES):
        wh = np.concatenate(
            [
                Wqkv[h * HD:(h + 1) * HD],                 # q rows
                Wqkv[C + h * HD:C + (h + 1) * HD],         # k rows
                Wqkv[2 * C + h * HD:2 * C + (h + 1) * HD], # v rows
            ],
            axis=0,
        )                                                  # [3*HD, C]
        wqkvT_h = np.ascontiguousarray(wh.T.astype(bf16))  # [C, 3*HD]
        in_maps.append({
            "xT": xT,
            "wqkvT": wqkvT_h,
            "wprojT": wprojT,
            "cosT": cosT,
            "sT": sT,
            "bias": bias2,
        })
    return in_maps


def assemble_output(results):
    # core h's out [C, 4*256]: column b*256+i -> global token b*S + h*256 + i
    y = np.empty((T, C), np.float32)
    for h in range(NCORES):
        o = np.asarray(results[h]["out"], np.float32).T  # [1024, C]
        for b in range(B):
            y[b * S + h * BSLICE:b * S + (h + 1) * BSLICE] = \
                o[b * BSLICE:(b + 1) * BSLICE]
    return y.reshape(B, S, C)


def kernel(x, cos, sin, Wqkv, Wproj, bproj, _trace=False, **run_kwargs):
    nc = _get_nc()
    in_maps = make_in_maps(x, cos, sin, Wqkv, Wproj, bproj)
    res = run_bass_kernel_spmd(
        nc, in_maps, core_ids=list(range(NCORES)), trace=_trace, **run_kwargs
    )
    out = assemble_output(res.results)
    kernel.last_results = res
    return out


if __name__ == "__main__":
    nc = build_nc()
    print("built OK, instructions:", len(nc.inst_map))


# revision 57
# speedup vs baseline: 1.0242x; 1.0242x over previous
"""Multi-head attention (B=4, S=2048, C=768, H=8, HD=96) on 8 TRN2 NeuronCores.

Strategy: tensor-parallel by head — one head per core. All TensorEngine
matmuls run bf16 inputs with f32 PSUM accumulation.

  - q/k are computed transposed: qT/kT [HD, tok] = W_chunk.T @ xT with the
    weight chunk stationary and 512-token moving operand; xT and all weights
    are host pre-transposed bf16.
  - v is computed directly in k-major layout (out [tok, HD] = xT_chunk.T @ Wv)
    so no DMA/PE transposes of v are needed; a ones column at hd index 96
    makes PV accumulate the softmax denominator in partition row 96.
  - RoPE runs in the transposed layout: the pair-swap is a DVE stream
    shuffle; the sign lives in the sin table (s[2i] = -sin[2i]).
  - Attention per (b, q-tile): scores.T [k,q] = kT.T @ qT on PE, exp on ACT
    (scale folded in; no max-subtraction needed: scores ~ N(0,1)), P.T (bf16)
    feeds PV directly: acc [HD+1, q] = v_aug.T @ P.T.
  - STRIDED q-tiles: q-tile g covers tokens {256j + 64g + t : j<8, t<64} of
    its batch (RoPE scatter-writes a qt-major buffer), so each q-tile
    completes a full 64-token column slice of the exchange payload for every
    destination. Batches 0-2 exchange once per batch; batch 3 in two halves,
    so the tail only exposes a half-size collective.
  - A dummy 1KB AllToAll fires at t=0: the CC stream's ~30-65us cold start
    runs under the batch-0 prologue instead of wedging the first exchange.
  - Normalized outputs stage in SBUF ([hd, dst, qt, t]) and store with one
    contiguous 512B-run DMA per batch — fragmented exchange buffers slow the
    CC-core reads by 2-3x.
  - exp runs N=1024 (one ACTIVATE per pair of score banks), amortizing the
    ~352-cycle ACT instruction overhead.
  - Normalization: approx-reciprocal of the denominator row on DVE, then a
    partition_broadcast on the otherwise-idle Pool engine (keeping it out of
    the PE/PSUM path avoids ring couplings that convoy the pipeline).
  - QKV matmuls for batch b+1 (and hoisted tail projections) are injected as
    PE "fillers" between the score/PV matmuls of batch b so the PE never
    idles waiting for exp. Projections run at the tail, 2-bank interleaved;
    proj(3) goes in column halves so the first half hoists into batch-3
    attention as soon as its half-collective lands.
  - PSUM (8 banks): score pairs 2x[128,2,512] + PV acc 2 + qkv 2 (q/k/v
    chains time-share via the pool ring).
  - Queue discipline: agc loads ride gpsimd (a collective-done wait on Sync
    wedges the store stream and stalls the pipeline through the tile rings);
    proj outputs go out via the ACT queue. The prologue keeps the DMA
    instruction count on the critical path minimal (~0.6-0.75us issue cost
    per DMA instruction).
"""

import numpy as np
from contextlib import ExitStack

import concourse.bass as bass
from concourse import bacc
import concourse.tile as tile
from concourse import mybir
from concourse.bass_utils import run_bass_kernel_spmd

B, S, C, H, HD = 4, 2048, 768, 8, 96
T = B * S            # 8192 tokens
NCORES = 8
TSLICE = T // NCORES  # 1024 tokens per core for the projection
BSLICE = S // NCORES  # 256 tokens per (core, batch)
KC = C // 128        # 6 contraction chunks of 128
F32 = mybir.dt.float32
BF16 = mybir.dt.bfloat16


def build_nc():
    nc = bacc.Bacc(None, num_devices=NCORES)

    xT = nc.declare_dram_parameter("xT", [C, T], BF16, isOutput=False)
    wqkvT = nc.declare_dram_parameter("wqkvT", [C, 3 * HD], BF16, isOutput=False)
    wprojT = nc.declare_dram_parameter("wprojT", [C, C], BF16, isOutput=False)
    cosT = nc.declare_dram_parameter("cosT", [HD, S], BF16, isOutput=False)
    sT = nc.declare_dram_parameter("sT", [HD, S], BF16, isOutput=False)
    biasd = nc.declare_dram_parameter("bias", [128, KC], F32, isOutput=False)
    outd = nc.declare_dram_parameter("out", [C, TSLICE], BF16, isOutput=True)

    # exchange buffers: [8 dst blocks x HD, tokens]. Batches 0-2 exchange
    # once per batch (their collectives hide under compute); batch 3 goes in
    # two halves so the final exposed collective is half-sized. Each strided
    # q-tile writes the 64-token column slice it completes.
    a2a_in = [nc.dram_tensor(f"a2a_in{b}", [C, BSLICE], BF16) for b in range(3)]
    a2a_out = [nc.dram_tensor(f"a2a_out{b}", [C, BSLICE], BF16) for b in range(3)]
    # batch 3 splits 192+64 so the only tail-exposed collective carries 64
    # tokens (~98KB); batch 0 splits 128+128 so its exchange completes ~20us
    # earlier and the agc(0) collective-done wait can never wedge the gpsimd
    # queue mid-pipeline. More chunks than that lose: each collective has a
    # ~7us floor and the chunks serialize on the CC stream.
    B3W = (192, 64)
    a2a_in3 = [nc.dram_tensor(f"a2a_in3h{i}", [C, w], BF16)
               for i, w in enumerate(B3W)]
    a2a_out3 = [nc.dram_tensor(f"a2a_out3h{i}", [C, w], BF16)
                for i, w in enumerate(B3W)]
    a2a_in0 = [[nc.dram_tensor(f"a2a_in{b}h{i}", [C, 128], BF16)
                for i in range(2)] for b in range(3)]
    a2a_out0 = [[nc.dram_tensor(f"a2a_out{b}h{i}", [C, 128], BF16)
                 for i in range(2)] for b in range(3)]
    # full-size warmup: a 1KB dummy doesn't establish the large-transfer
    # path, and the first real exchange then still runs 2-3x slow
    warm_in = nc.dram_tensor("warm_in", [C, BSLICE], BF16)
    warm_out = nc.dram_tensor("warm_out", [C, BSLICE], BF16)

    SCALE = HD ** -0.5
    MULT = mybir.AluOpType.mult
    ADD = mybir.AluOpType.add
    EXP = mybir.ActivationFunctionType.Exp

    with tile.TileContext(nc, num_cores=NCORES) as tc, ExitStack() as ctx:
        const = ctx.enter_context(tc.tile_pool(name="const", bufs=1))
        # bufs=2 doubles as the prologue bandwidth gate: groups 2-3 (and
        # every later prefetch) queue behind consumption instead of
        # round-robin-stealing HBM bandwidth from the group-0/1 loads that
        # gate the first matmuls
        xtp = ctx.enter_context(tc.tile_pool(name="xtp", bufs=2))
        rawp = ctx.enter_context(tc.tile_pool(name="rawp", bufs=2))
        ropep = ctx.enter_context(tc.tile_pool(name="ropep", bufs=2))
        Pp = ctx.enter_context(tc.tile_pool(name="Pp", bufs=4))
        rcp = ctx.enter_context(tc.tile_pool(name="rcp", bufs=2))
        yp = ctx.enter_context(tc.tile_pool(name="yp", bufs=2))
        agcp = ctx.enter_context(tc.tile_pool(name="agcp", bufs=4))

        # PSUM (8 banks): qkv 2 + score pairs 2x2 + PV acc 2. The norm
        # broadcast borrows a slot in the score ring between q-tiles.
        psqkv = ctx.enter_context(tc.tile_pool(name="psqkv", bufs=2, space="PSUM"))
        pssc = ctx.enter_context(tc.tile_pool(name="pssc", bufs=2, space="PSUM"))
        psacc = ctx.enter_context(tc.tile_pool(name="psacc", bufs=2, space="PSUM"))

        # dummy collective with no input deps, fired at t=0: the CC stream
        # pays its ~30us cold-start here, under the batch-0 prologue, so the
        # first real exchange runs at the steady ~8us and nothing queued
        # behind its done-semaphore wedges mid-pipeline
        nc.gpsimd.collective_compute(
            "AllToAll", mybir.AluOpType.bypass,
            replica_groups=[list(range(NCORES))],
            ins=[warm_in.ap().opt()],
            outs=[warm_out.ap().opt()],
        )

        # --- constants --- (wq + first x groups first: they gate the PE
        # start; the DMA rings drain all queued transfers round-robin, so
        # everything not needed immediately is issued later / behind an
        # xtp-slot gate)
        # every DMA instruction costs ~0.6-0.75us of queue-issue time, so the
        # prologue keeps the count on the critical path minimal: kc0 of the
        # weights and of x group 0 first (2 small DMAs), then the rest whole
        wq_sb = const.tile([128, KC, 3 * HD], BF16)
        wqv = wqkvT.ap().rearrange("(kc p) n -> p kc n", p=128)
        nc.sync.dma_start(wq_sb[:, 0, :], wqv[:, 0, :])
        xTv = xT.ap().rearrange("(kc p) t -> p kc t", p=128)  # [128, KC, T]
        cosT_sb = const.tile([HD, S], BF16)
        sT_sb = const.tile([HD, S], BF16)
        wp_sb = const.tile([128, KC, C], BF16)
        bias_sb = const.tile([128, KC], F32)

        def load_tail_consts():
            nc.scalar.dma_start(
                wp_sb, wprojT.ap().rearrange("(kc p) n -> p kc n", p=128))
            nc.scalar.dma_start(bias_sb, biasd.ap())

        # persistent ping/pong per-batch qT stored qt-MAJOR ([HD, qt, 512]
        # with tile qt holding tokens {256j + 64qt + t} in (j, t) column
        # order — RoPE scatter-writes it), per-group kT, and k-major v with
        # a ones column at hd index 96 (denominator accumulator).
        qT = [const.tile([HD, 4, 512], BF16, name=f"qT{i}") for i in range(2)]
        kT = [[const.tile([HD, 512], BF16, name=f"kT{i}g{g}") for g in range(4)]
              for i in range(2)]
        vA = [[const.tile([128, 4, HD + 1], BF16, name=f"vA{i}g{g}")
               for g in range(4)] for i in range(2)]
        for i in range(2):
            for g in range(4):
                nc.vector.memset(vA[i][g][:, :, HD:HD + 1], 1.0)

        ones_sb = const.tile([1, HD], BF16, name="ones_sb")
        nc.vector.memset(ones_sb, 1.0)

        # per-batch normalized-output staging [hd, dst j, qt, t]: each qt's
        # norm writes its strided slice; one contiguous 512B-run store per
        # batch (b3: per half) keeps the exchange buffers DRAM-friendly
        stage = [const.tile([HD, 8, 4, 64], BF16, name=f"stage{i}")
                 for i in range(2)]

        SWAPMASK = []
        for i in range(16):
            SWAPMASK += [2 * i + 1, 2 * i]

        def load_xtc(b, g, split=False, eng=None):
            # split=True (prologue group 0): kc0 alone, then the rest — the
            # first matmuls start after ~0.6MB instead of the full group
            tok0 = b * S + g * 512
            xtc = xtp.tile([128, KC, 512], BF16, tag="xtc")
            e = eng or nc.sync
            if split:
                e.dma_start(xtc[:, 0, :], xTv[:, 0, tok0:tok0 + 512])
                e.dma_start(xtc[:, 1:, :], xTv[:, 1:, tok0:tok0 + 512])
            else:
                e.dma_start(xtc, xTv[:, :, tok0:tok0 + 512])
            return xtc

        def qkv_fillers(b, xtcs, alt=False):
            """Filler closures computing q/k/v for batch b (4 groups).

            With 2 qkv PSUM banks the chains time-share the pool ring:
            q(slot a), k(slot b), then v waits for rope-q's copy to free
            slot a; the next group's q waits for rope-k's copy, etc.
            alt=True (batch-0 prologue only): odd groups borrow the then-idle
            score banks so two group-chains run in parallel.
            """
            out = []
            q_b, k_b, v_b = qT[b % 2], kT[b % 2], vA[b % 2]
            for g in range(4):
                state = {}
                xtc = xtcs[g]
                seq = slice(g * 512, (g + 1) * 512)
                pool, ptag = ((pssc, "sc") if (alt and g % 2 == 1)
                              else (psqkv, "qkv"))

                def alloc_qk(state=state, pool=pool, ptag=ptag):
                    state["q"] = pool.tile([HD, 512], F32, tag=ptag, name="psq")
                    state["k"] = pool.tile([HD, 512], F32, tag=ptag, name="psk")

                def alloc_v(state=state, pool=pool, ptag=ptag):
                    state["v"] = pool.tile([128, 4, HD], F32, tag=ptag, name="psv")

                def mm_qk(kc, which, state=state, xtc=xtc):
                    nc.tensor.matmul(
                        state[which],
                        wq_sb[:, kc, (0 if which == "q" else HD):(HD if which == "q" else 2 * HD)],
                        xtc[:, kc, :],
                        start=(kc == 0), stop=(kc == KC - 1),
                    )

                def mm_v(kc, c, state=state, xtc=xtc):
                    nc.tensor.matmul(
                        state["v"][:, c, :],
                        xtc[:, kc, c * 128:(c + 1) * 128],
                        wq_sb[:, kc, 2 * HD:3 * HD],
                        start=(kc == 0), stop=(kc == KC - 1),
                    )

                def rope(which, dst, dst_scatter, state=state, seq=seq):
                    # bf16 intermediates keep the two multiplies and the add
                    # in the DVE 2x perf mode (fp32 outputs force 1x)
                    ps = state[which]
                    raw = rawp.tile([HD, 512], BF16, tag="raw")
                    nc.vector.tensor_copy(out=raw, in_=ps)
                    rot = rawp.tile([HD, 512], BF16, tag="rot")
                    nc.vector.stream_shuffle(rot, raw, SWAPMASK)
                    t1 = ropep.tile([HD, 512], BF16, tag="t1")
                    nc.vector.tensor_tensor(t1, raw, cosT_sb[:, seq], MULT)
                    t2 = ropep.tile([HD, 512], BF16, tag="t2")
                    nc.vector.tensor_tensor(t2, rot, sT_sb[:, seq], MULT)
                    if dst_scatter:
                        t1 = t1.rearrange("hd (j q t) -> hd j q t", j=2, t=64)
                        t2 = t2.rearrange("hd (j q t) -> hd j q t", j=2, t=64)
                    nc.vector.tensor_tensor(dst, t1, t2, ADD)

                def copy_v(g=g, state=state, v_b=v_b):
                    for c in range(4):
                        nc.vector.tensor_copy(
                            out=v_b[g][:, c, 0:HD], in_=state["v"][:, c, :])

                out.append(alloc_qk)
                for kc in range(KC):
                    out.append(lambda kc=kc, f=mm_qk: f(kc, "q"))
                    out.append(lambda kc=kc, f=mm_qk: f(kc, "k"))
                # rope-q scatter-writes the qt-major buffer: group g's
                # token col (jr, qt, t) -> tile qt, col 64*(2g+jr) + t.
                # Its first copy frees the q PSUM slot for v.
                qdst = q_b.rearrange("hd q (j t) -> hd j q t", j=8, t=64) \
                    [:, 2 * g:2 * g + 2, :, :]
                out.append(lambda f=rope, qdst=qdst: f("q", qdst, True))
                out.append(alloc_v)
                # v regions share one PSUM bank: each chunk c must fully
                # accumulate (start..stop) before the next chunk's start
                vs = []
                for c in range(4):
                    for kc in range(KC):
                        vs.append(lambda kc=kc, c=c, f=mm_v: f(kc, c))
                vs.append(lambda f=rope, k_b=k_b, g=g: f("k", k_b[g][:, :], False))
                out.extend(vs)
                out.append(copy_v)
            return out

        def load_agc_only(b, eng=None):
            # always gpsimd unless told otherwise: a collective-done wait on
            # the Sync queue convoys the per-tile a2a stores behind it and
            # stalls the whole pipeline through the onorm tile ring.
            agc = agcp.tile([128, KC, BSLICE], BF16, tag="agc", name="agc")
            e = eng or nc.gpsimd
            if b < 3:
                for i in range(2):
                    e.dma_start(
                        agc[:, :, 128 * i:128 * (i + 1)],
                        a2a_out0[b][i].ap().rearrange("(kc p) t -> p kc t", p=128))
            else:
                t0 = 0
                for i, w in enumerate(B3W):
                    e.dma_start(
                        agc[:, :, t0:t0 + w],
                        a2a_out3[i].ap().rearrange("(kc p) t -> p kc t", p=128))
                    t0 += w
            return agc

        def proj_fillers(b, wide=False, agc_pre=None, cols=(0, BSLICE)):
            """Filler closures projecting batch b's gathered attention output.

            wide=True (tail only): interleave 2 ko-chunk accumulators across
            the idle qkv PSUM banks so consecutive matmuls never share a
            bank. Bias-add + f32 copy runs on DVE to keep ACT free for exp.
            cols: token sub-range — batch 3 projects in halves so the first
            half hoists into batch-3 attention once its collective lands.
            """
            out = []
            state = {}
            t0, t1 = cols
            tsz = t1 - t0

            def load_agc():
                # tail-only path: the ACT queue is safe here (all exps are
                # past; only this proj's own downstream sits behind it), and
                # HWDGE issue is ~6-10us faster than SWDGE descriptor gen
                state["agc"] = agc_pre if agc_pre is not None else \
                    load_agc_only(b, eng=nc.scalar)
                state["y"] = yp.tile([128, KC, tsz], BF16, tag="y", name="ysb")

            def alloc_py(ko, pool, name):
                state[ko] = pool.tile([128, tsz], F32, tag="qkv" if pool is psqkv else "py", name=name)

            def mm(ko, kc):
                nc.tensor.matmul(
                    state[ko], wp_sb[:, kc, ko * 128:(ko + 1) * 128],
                    state["agc"][:, kc, t0:t1],
                    start=(kc == 0), stop=(kc == KC - 1),
                )

            def biascopy(ko):
                nc.vector.tensor_scalar_add(
                    state["y"][:, ko, :], state[ko], bias_sb[:, ko:ko + 1])

            def store():
                nc.scalar.dma_start(
                    outd.ap().rearrange("(kc p) t -> p kc t", p=128)
                    [:, :, b * BSLICE + t0:b * BSLICE + t1],
                    state["y"])

            out.append(load_agc)
            if wide:
                for grp in range(3):
                    kos = [2 * grp + i for i in range(2)]
                    for ko in kos:
                        out.append(lambda ko=ko: alloc_py(ko, psqkv, "pyw"))
                    for kc in range(KC):
                        for ko in kos:
                            out.append(lambda ko=ko, kc=kc: mm(ko, kc))
                    for ko in kos:
                        out.append(lambda ko=ko: biascopy(ko))
            else:
                for ko in range(KC):
                    out.append(lambda ko=ko: alloc_py(ko, pspy, "py"))
                    for kc in range(KC):
                        out.append(lambda ko=ko, kc=kc: mm(ko, kc))
                    out.append(lambda ko=ko: biascopy(ko))
            out.append(store)
            return out

        def attention_tile(b, qt, fillers):
            q_b, k_b, v_b = qT[b % 2], kT[b % 2], vA[b % 2]
            # qt-major strided q-tile: tokens {256j + 64*qt + t}, (j,t) order
            qmv = q_b[:, qt, :]

            def emit(n):
                for _ in range(min(n, len(fillers))):
                    fillers.pop(0)()

            acc = psacc.tile([HD + 1, 512], F32, tag="acc")
            Pts = [None] * 8
            for i in range(8):
                sc2 = pssc.tile([128, 2, 512], F32, tag="sc")
                for h2 in range(2):
                    kt = 2 * i + h2
                    nc.tensor.matmul(
                        sc2[:, h2, :],
                        k_b[kt // 4][:, (kt % 4) * 128:(kt % 4 + 1) * 128],
                        qmv,
                        start=True, stop=True,
                    )
                # one exp instruction covers both score banks: N=1024
                # amortizes the ~352-cycle ACT instruction overhead
                Pt2 = Pp.tile([128, 2, 512], BF16, tag="Pt")
                nc.scalar.activation(
                    Pt2.rearrange("p two n -> p (two n)"),
                    sc2.rearrange("p two n -> p (two n)"), EXP, scale=SCALE)
                Pts[i] = Pt2
                if i > 0:
                    for h2 in range(2):
                        kt = 2 * (i - 1) + h2
                        nc.tensor.matmul(
                            acc, v_b[kt // 4][:, kt % 4, :], Pts[i - 1][:, h2, :],
                            start=(kt == 0), stop=False,
                        )
                emit(8)
            for h2 in range(2):
                kt = 14 + h2
                nc.tensor.matmul(
                    acc, v_b[3][:, kt % 4, :], Pts[7][:, h2, :],
                    start=False, stop=(kt == 15),
                )

            # normalize: approx-reciprocal of the denominator row (single
            # custom-DVE op, ~18 bits), broadcast across partitions on the
            # otherwise-idle Pool engine. Keeping the broadcast out of the
            # score PSUM ring matters: a bc tile there couples next-tile
            # scores to the DVE queue depth and convoys the whole pipeline.
            dnrow = rcp.tile([1, 512], F32, tag="dnrow")
            nc.vector.tensor_copy(out=dnrow, in_=acc[HD:HD + 1, :])
            rc = rcp.tile([1, 512], F32, tag="rc")
            nc.vector.reciprocal_approx_fast(out=rc, in_=dnrow)
            bcs = rcp.tile([HD, 512], F32, tag="bcs", name="bcs")
            nc.gpsimd.partition_broadcast(bcs, rc, channels=HD)
            # normalized tile lands in the staging buffer at its qt slice,
            # [hd, j, t] (strided DVE write)
            stg = stage[b % 2]
            nc.vector.tensor_tensor(
                stg[:, :, qt, :],
                acc[0:HD, :].rearrange("hd (j t) -> hd j t", t=64),
                bcs.rearrange("hd (j t) -> hd j t", t=64), MULT)
            # Batches 0-2: two halves each (early completion keeps the agc
            # collective-done waits from wedging the gpsimd queue, and no
            # large late block can delay batch 3's chunks on the CC stream).
            # Batch 3: 192+64 (small exposed tail).
            chunk = None
            if b < 3 and qt in (1, 3):
                chunk = (a2a_in0[b][qt // 2], a2a_out0[b][qt // 2],
                         (0, 2) if qt == 1 else (2, 4))
            elif b == 3 and qt in (2, 3):
                chunk = (a2a_in3[qt - 2], a2a_out3[qt - 2],
                         (0, 3) if qt == 2 else (3, 4))
            if chunk is not None:
                cin, cout, (g0, g1) = chunk
                nc.sync.dma_start(
                    cin.ap().rearrange("(j hd) t -> hd j t", hd=HD),
                    stg[:, :, g0:g1, :].rearrange("hd j g t -> hd j (g t)"))
                nc.gpsimd.collective_compute(
                    "AllToAll", mybir.AluOpType.bypass,
                    replica_groups=[list(range(NCORES))],
                    ins=[cin.ap().opt()],
                    outs=[cout.ap().opt()],
                )

        # prologue: batch 0 qkv straight-line; x groups ride the sync and
        # scalar DMA rings in parallel so the first matmuls start as soon as
        # group 0 chunk 0 lands. cos/sin lead the scalar ring: rope(g0)
        # needs them ~6us in, and a late arrival stalls the 2 qkv PSUM banks.
        nc.scalar.dma_start(cosT_sb, cosT.ap())
        nc.scalar.dma_start(sT_sb, sT.ap())
        xtcs0 = [load_xtc(0, 0, split=True)]
        nc.sync.dma_start(wq_sb[:, 1:, :], wqv[:, 1:, :])
        xtcs0.append(load_xtc(0, 1, eng=nc.scalar))
        xtcs0.append(load_xtc(0, 2))
        xtcs0.append(load_xtc(0, 3, eng=nc.scalar))
        load_tail_consts()
        for f in qkv_fillers(0, xtcs0):
            f()

        # all projections run at the tail (3-bank wide): their agc loads sit
        # on the gpsimd queue after every trigger, so collective-done waits
        # can never convoy per-tile work
        PROJ_IN = {}
        agc_pre = {}
        for b in range(B):
            fillers = []
            if b + 1 < B:
                xtcs = [load_xtc(b + 1, g) for g in range(4)]
                fillers += qkv_fillers(b + 1, xtcs)
            for pb in PROJ_IN.get(b, []):
                fillers += proj_fillers(pb)
            for qt in range(4):
                attention_tile(b, qt, fillers)
                if b == B - 1 and qt == 0:
                    # preload so the scheduler can hoist tail projections
                    # into batch 3's exp-paced PE gaps. Both ride gpsimd (a
                    # collective wait on Sync wedges the store stream), and
                    # only after batch 3's first tile so their waits sit
                    # behind that broadcast — never ahead of live per-tile
                    # work while an old collective is still in flight.
                    agc_pre[0] = load_agc_only(0)
                    agc_pre[1] = load_agc_only(1)
            for f in fillers:
                f()
        # tail: proj(0..2) overlap the last collectives; proj(3) goes in
        # halves so its first 128 tokens hoist into batch-3 attention as
        # soon as the first half-collective lands. All 2-bank interleaved
        # across the now-idle qkv PSUM banks.
        for pb in range(3):
            for f in proj_fillers(pb, wide=True, agc_pre=agc_pre.get(pb)):
                f()
        agc3 = load_agc_only(3, eng=nc.scalar)
        for cols in ((0, 192), (192, 256)):
            for f in proj_fillers(3, wide=True, agc_pre=agc3, cols=cols):
                f()

    nc.compile()
    return nc


_NC_CACHE = None


def _get_nc():
    global _NC_CACHE
    if _NC_CACHE is None:
        _NC_CACHE = build_nc()
    return _NC_CACHE


def make_in_maps(x, cos, sin, Wqkv, Wproj, bproj):
    import ml_dtypes

    bf16 = ml_dtypes.bfloat16
    x = np.asarray(x, np.float32)
    cos = np.asarray(cos, np.float32)
    sin = np.asarray(sin, np.float32)
    Wqkv = np.asarray(Wqkv, np.float32)
    Wproj = np.asarray(Wproj, np.float32)
    bproj = np.asarray(bproj, np.float32)

    xT = np.ascontiguousarray(x.reshape(T, C).T.astype(bf16))  # [C, T] bf16
    wprojT = np.ascontiguousarray(Wproj.T.astype(bf16))        # [C_in, C_out]
    s = sin.copy()
    s[:, 0::2] = -sin[:, 0::2]
    cosT = np.ascontiguousarray(cos.T.astype(bf16))            # [HD, S] bf16
    sT = np.ascontiguousarray(s.T.astype(bf16))                # [HD, S] bf16
    bias2 = np.ascontiguousarray(bproj.reshape(KC, 128).T)     # [128, KC]

    in_maps = []
    for h in range(NCORES):
        wh = np.concatenate(
            [
                Wqkv[h * HD:(h + 1) * HD],                 # q rows
                Wqkv[C + h * HD:C + (h + 1) * HD],         # k rows
                Wqkv[2 * C + h * HD:2 * C + (h + 1) * HD], # v rows
            ],
            axis=0,
        )                                                  # [3*HD, C]
        wqkvT_h = np.ascontiguousarray(wh.T.astype(bf16))  # [C, 3*HD]
        in_maps.append({
            "xT": xT,
            "wqkvT": wqkvT_h,
            "wprojT": wprojT,
            "cosT": cosT,
            "sT": sT,
            "bias": bias2,
        })
    return in_maps


def assemble_output(results):
    # core h's out [C, 4*256]: column b*256+i -> global token b*S + h*256 + i
    y = np.empty((T, C), np.float32)
    for h in range(NCORES):
        o = np.asarray(results[h]["out"], np.float32).T  # [1024, C]
        for b in range(B):
            y[b * S + h * BSLICE:b * S + (h + 1) * BSLICE] = \
                o[b * BSLICE:(b + 1) * BSLICE]
    return y.reshape(B, S, C)


def kernel(x, cos, sin, Wqkv, Wproj, bproj, _trace=False, **run_kwargs):
    nc = _get_nc()
    in_maps = make_in_maps(x, cos, sin, Wqkv, Wproj, bproj)
    res = run_bass_kernel_spmd(
        nc, in_maps, core_ids=list(range(NCORES)), trace=_trace, **run_kwargs
    )
    out = assemble_output(res.results)
    kernel.last_results = res
    return out


if __name__ == "__main__":
    nc = build_nc()
    print("built OK, instructions:", len(nc.inst_map))
